# revision 37
# baseline (speedup 1.0000x reference)
"""BiLSTM-CRF loss kernel for Trainium2, 8-core data parallel.

Per-core (batch shard of 32), feature-major ("transposed") layout throughout:
gates/features live on partitions, batch on the free dim, so every elementwise
op runs at 128-partition occupancy with a small free size.

  - z_t for each direction accumulates in PSUM as [128 gates-in-chunk,
    8 chunks, batch]: per (chunk, dir) group = 1 bias matmul (K=1 ones rhs)
    + 4 x-projection matmuls (xg in [E, token] layout, consumed in-loop; no
    DRAM z roundtrip) + 2 recurrent matmuls off the transposed h buffer.
  - one sigmoid covers all 8 gate chunks; the g-gate rows of W/b are
    host-prescaled by 2 so tanh(g) = 2*sigmoid(z_g) - 1, done as a single
    DVE scalar_tensor_tensor. f*c runs on GpSimd (Pool) off the DVE path.
  - h = sigma_o * tanh(c) is written directly into the persistent transposed
    h buffer [128, k, dir, token] feeding both the next step's matmuls and
    the emission matmuls -- no PE transposes anywhere.
  - emissions (em = Wout.[hf;hb]) are computed per 512-token block as soon
    as both chains have covered it, with exp/vmask/gold-dot fused in.
  - CRF partition function in scaled linear space with an absorbing 77th
    tag: meet-in-the-middle (alpha forward 64 steps, beta/gamma backward 64
    steps, run concurrently), Z = alpha_63 . (M gamma_64).
Host combines the 8 per-core partial sums into the scalar loss.
"""

import numpy as np
import ml_dtypes

import concourse.bass as bass
import concourse.mybir as mybir
from concourse.tile import TileContext
from concourse.vector_clock import ScopedClock
from concourse.alu_op_type import AluOpType as ALU

N_CORES = 8
B, S, E, HD, T, V = 256, 128, 512, 256, 76, 30000
BC = B // N_CORES          # 32 batch per core
G4 = 4 * HD                # 1024 gates per direction
TA = T + 1                 # 77 tags with absorber
NTOK = S * BC              # 4096 tokens per direction per core

dt = mybir.dt
F32, BF16 = dt.float32, dt.bfloat16
AF = mybir.ActivationFunctionType

# ---------------------------------------------------------------- tile patch
# This walrus build rejects >1 sem wait on CTRL-class (Drain/NoOp)
# instructions; split the Tile tail-drain waits across preceding NOPs.
_MAX_WAITS = 1
_WAIT_LIMITS = {}


def _split_excess_waits(nc):
    """Non-DMA instructions accept only one sem wait on this walrus build;
    move excess waits onto NOPs spliced in front (same engine, same order)."""
    for f in nc.m.functions:
        stack = list(f.blocks)
        while stack:
            bb = stack.pop()
            for sub in getattr(bb, "blocks", []) or []:
                stack.append(sub)
            insts = getattr(bb, "instructions", None)
            if not insts:
                continue
            newlist = []
            changed = False
            for inst in insts:
                si = inst.sync_info
                lim = _WAIT_LIMITS.get(type(inst).__name__, 1)
                if si is not None and si.on_wait and len(si.on_wait) > lim:
                    waits = list(si.on_wait)
                    si.on_wait = waits[-lim:]
                    for w in waits[:-lim]:
                        nop = mybir.InstNoOp(
                            name=f"I-wsplit{nc.next_id()}", ins=[], outs=[],
                            engine=inst.engine,
                            sync_info=mybir.SyncInfo(on_wait=[w], on_update=[]),
                        )
                        newlist.append(nop)
                    changed = True
                newlist.append(inst)
            if changed:
                insts[:] = newlist


def _patched_drain_and_barrier(self, tick_clock, wait_clock):
    nc = self.nc
    _split_excess_waits(nc)
    nops = [nc.sync.nop(nofuse=True, hint=f"waitsplit{i}") for i in range(16)]
    drain_inst = nc.sync.drain()
    wait_clock.add_sem_waits(
        drain_inst.ins, ScopedClock({None: tick_clock.global_clock})
    )
    si = drain_inst.ins.sync_info
    if si is not None and si.on_wait and len(si.on_wait) > _MAX_WAITS:
        waits = list(si.on_wait)
        chunks = [waits[i:i + _MAX_WAITS] for i in range(0, len(waits), _MAX_WAITS)]
        si.on_wait = chunks[-1]
        assert len(chunks) - 1 <= len(nops), "too many wait chunks"
        for i, ch in enumerate(chunks[:-1]):
            ni = nops[i].ins
            if ni.sync_info is None:
                ni.sync_info = mybir.SyncInfo(on_wait=ch, on_update=[])
            else:
                ni.sync_info.on_wait = list(ni.sync_info.on_wait) + ch
    nc.all_engine_barrier()
    assert self.sems is not None
    popped = nc._tile_sem_poison_stack.pop()
    assert popped is self._sem_poison
    allsems = list(self.sems.allocated().values())
    for i in range(0, len(allsems), 8):
        nc.clear_and_free_semaphores(allsems[i:i + 8])
    nc.all_engine_barrier()


def apply_tile_patch():
    TileContext._drain_and_barrier = _patched_drain_and_barrier


# ---------------------------------------------------------------- builder
def build_nc():
    apply_tile_patch()
    nc = bass.Bass("TRN2", target_bir_lowering=False, debug=False,
                   num_devices=N_CORES)

    xt_d = nc.dram_tensor("xt", [2, 128, 4, NTOK], BF16, kind="ExternalInput")
    wih_d = nc.dram_tensor("wih", [128, 2, 4, G4], BF16, kind="ExternalInput")
    whh_d = nc.dram_tensor("whh", [128, 2, 2, G4], BF16, kind="ExternalInput")
    bias_d = nc.dram_tensor("biast", [1, 2, 8, 128], BF16, kind="ExternalInput")
    wout_d = nc.dram_tensor("wout", [128, 2, 2, T], BF16, kind="ExternalInput")
    h0t_d = nc.dram_tensor("h0t", [128, 2, 2, BC], BF16, kind="ExternalInput")
    c0t_d = nc.dram_tensor("c0t", [128, 2, 2, BC], F32, kind="ExternalInput")
    mp_d = nc.dram_tensor("mp", [TA, TA], BF16, kind="ExternalInput")
    mpt_d = nc.dram_tensor("mpt", [TA, TA], BF16, kind="ExternalInput")
    eend_d = nc.dram_tensor("eend", [TA, 1], F32, kind="ExternalInput")
    bvec_d = nc.dram_tensor("bvec", [T, 2], F32, kind="ExternalInput")
    ohm_d = nc.dram_tensor("ohm", [T, NTOK], BF16, kind="ExternalInput")
    vmask_d = nc.dram_tensor("vmask", [T, NTOK], BF16, kind="ExternalInput")
    padrow_d = nc.dram_tensor("padrow", [1, NTOK], BF16, kind="ExternalInput")
    out_d = nc.dram_tensor("out", [1, 2], F32, kind="ExternalOutput")

    NB = S // 16  # 8 emission blocks of 512 tokens
    # slot (0-based) after which emission block b is fully available; the
    # backward chain is software-pipelined two slots behind the forward one
    em_ready = {}
    for b in range(NB):
        r = max(16 * b + 15, S - 1 - 16 * b + 2)
        em_ready.setdefault(r, []).append(b)

    with TileContext(nc) as tc:
        with (
            tc.tile_pool(name="const", bufs=1) as cpool,
            tc.tile_pool(name="hbuf", bufs=1) as hpool,
            tc.tile_pool(name="gate", bufs=3) as gpool,
            tc.tile_pool(name="cell", bufs=3) as spool,
            tc.tile_pool(name="work", bufs=3) as wpool,
            tc.tile_pool(name="zps", bufs=2, space="PSUM") as zps_pool,
            tc.tile_pool(name="emps", bufs=1, space="PSUM") as emps_pool,
            tc.tile_pool(name="crfps", bufs=2, space="PSUM") as crfps_pool,
        ):
            # ---- constants / weights into SBUF.  DMA order is the startup
            # critical path: everything step 0 needs (bias, h0, c0, wih,
            # whh, first xg chunk) goes first; the rest streams in behind.
            bias_sb = cpool.tile([1, 2, 8, 128], BF16)
            nc.sync.dma_start(bias_sb[:], bias_d[:])
            h0t_sb = cpool.tile([128, 2, 2, BC], BF16)
            nc.sync.dma_start(h0t_sb[:], h0t_d[:])
            c0t_sb = cpool.tile([128, 2, 2, BC], F32)
            nc.sync.dma_start(c0t_sb[:], c0t_d[:])
            wih_sb = cpool.tile([128, 2, 4, G4], BF16)
            nc.sync.dma_start(wih_sb[:, 0], wih_d.ap()[:, 0])
            whh_sb = cpool.tile([128, 2, 2, G4], BF16)
            nc.sync.dma_start(whh_sb[:], whh_d[:])
            xg = {d: hpool.tile([128, 4, NTOK], BF16, name=f"xg{d}")
                  for d in range(2)}
            NCH = 4
            CW = NTOK // NCH
            nc.sync.dma_start(xg[0][:, :, 0:CW], xt_d.ap()[0, :, :, 0:CW])
            nc.sync.dma_start(wih_sb[:, 1], wih_d.ap()[:, 1])
            nc.sync.dma_start(xg[1][:, :, 0:CW], xt_d.ap()[1, :, :, 0:CW])

            wout_sb = cpool.tile([128, 2, 2, T], BF16)
            nc.sync.dma_start(wout_sb[:], wout_d[:])
            mp_sb = cpool.tile([TA, TA], BF16)
            nc.sync.dma_start(mp_sb[:], mp_d[:])
            mpt_sb = cpool.tile([TA, TA], BF16)
            nc.sync.dma_start(mpt_sb[:], mpt_d[:])
            eend_sb = cpool.tile([TA, 1], F32)
            nc.sync.dma_start(eend_sb[:], eend_d[:])
            bvec_sb = cpool.tile([T, 2], F32)
            nc.sync.dma_start(bvec_sb[:], bvec_d[:])

            ones1 = cpool.tile([1, BC], BF16)
            nc.vector.memset(ones1[:], 1.0)
            onesd = cpool.tile([128, 2, BC], BF16)
            nc.vector.memset(onesd[:], 1.0)
            ones77 = cpool.tile([TA, 1], F32)
            nc.vector.memset(ones77[:], 1.0)

            # remaining xg chunks stream in behind the first ones
            for c in range(1, NCH):
                for d in range(2):
                    nc.sync.dma_start(
                        xg[d][:, :, c * CW:(c + 1) * CW],
                        xt_d.ap()[d, :, :, c * CW:(c + 1) * CW])
            # transposed h, one tile per direction: [128, k-chunk, token]
            hts = {d: hpool.tile([128, 2, NTOK], BF16, name=f"hts{d}")
                   for d in range(2)}
            # emissions (scaled-exp'd), bf16, absorber row 76
            em_sb = hpool.tile([TA, NTOK], BF16, name="em")
            ohm_sb = hpool.tile([T, NTOK], BF16, name="ohm")
            nc.sync.dma_start(ohm_sb[:], ohm_d[:])
            vm_sb = hpool.tile([T, NTOK], BF16, name="vm")
            nc.sync.dma_start(vm_sb[:], vmask_d[:])
            nc.sync.dma_start(em_sb[T:TA, :], padrow_d[:])

            # ---- z PSUM tile helpers -----------------------------------
            def emit_bias_x(zt, d, t):
                """bias + x-projection matmuls of direction d for step t into
                PSUM tile zt [128, 8 gate-chunk, BC]."""
                tok = slice(t * BC, (t + 1) * BC)
                for gc in range(8):
                    nc.tensor.matmul(
                        zt[:, gc, :], bias_sb[:, d, gc, :],
                        ones1[:], start=True, stop=False)
                for ek in range(4):
                    for gc in range(8):
                        nc.tensor.matmul(
                            zt[:, gc, :],
                            wih_sb[:, d, ek, gc * 128:(gc + 1) * 128],
                            xg[d][:, ek, tok], start=False, stop=False)

            def emit_h(zt, d, t):
                """recurrent matmuls (Whh . h_{t-1}) closing step t's groups."""
                for k in range(2):
                    if t == 0:
                        hk = h0t_sb[:, d, k, :]
                    else:
                        col = (t - 1 if d == 0 else S - t) * BC
                        hk = hts[d][:, k, col:col + BC]
                    for gc in range(8):
                        nc.tensor.matmul(
                            zt[:, gc, :],
                            whh_sb[:, d, k, gc * 128:(gc + 1) * 128],
                            hk, start=False, stop=(k == 1))

            # ---- emission block -----------------------------------------
            em_accs = []
            deferred_red = []

            def emit_emission(b, late):
                blk = slice(b * 512, (b + 1) * 512)
                ps = emps_pool.tile([T, 512], F32, tag="emps")
                i = 0
                for d in range(2):
                    for k in range(2):
                        nc.tensor.matmul(ps[:], wout_sb[:, k, d, :],
                                         hts[d][:, k, blk],
                                         start=(i == 0), stop=(i == 3))
                        i += 1
                # scaled emissions first: exp(em + b_out [+ start on col 0])
                if b == 0:
                    nc.scalar.activation(em_sb[0:T, 0:BC], ps[:, 0:BC],
                                         AF.Exp, bias=bvec_sb[:, 1:2])
                    nc.scalar.activation(em_sb[0:T, BC:512], ps[:, BC:512],
                                         AF.Exp, bias=bvec_sb[:, 0:1])
                else:
                    nc.scalar.activation(em_sb[0:T, blk], ps[:],
                                         AF.Exp, bias=bvec_sb[:, 0:1])
                nc.vector.tensor_mul(em_sb[0:T, blk], em_sb[0:T, blk],
                                     vm_sb[:, blk])
                # gold-path dot on raw em (b_out part handled on host); the
                # reduce of end-of-sequence blocks runs inside the CRF span
                scr = wpool.tile([T, 512], BF16, tag=f"scr{b}", bufs=1,
                                 name=f"scr{b}")
                nc.vector.tensor_mul(scr[:], ps[:], ohm_sb[:, blk])
                acc = wpool.tile([T, 1], F32, tag=f"emacc{b}", bufs=1,
                                 name=f"emacc{b}")
                em_accs.append(acc)
                nc.vector.tensor_reduce(acc[:], scr[:],
                                        axis=mybir.AxisListType.X,
                                        op=ALU.add)

            # ---- LSTM loop ----------------------------------------------
            # Forward chain runs in slot t = its step t; the backward chain
            # is software-pipelined one slot behind (step t in slot t+1) so
            # its Act/DVE ops always have ready inputs and can never stall
            # the forward chain through the in-order engine queues.
            c_st = {d: c0t_sb[:, d, :, :] for d in range(2)}

            def sig_phase(d, zt):
                g = gpool.tile([128, 8, BC], BF16, tag=f"g{d}", name=f"g{d}")
                nc.scalar.activation(g[:], zt[:], AF.Sigmoid)
                return g

            def dve_phase(d, g):
                fc = spool.tile([128, 2, BC], F32, tag=f"fc{d}",
                                name=f"fc{d}")
                nc.gpsimd.tensor_mul(fc[:], g[:, 2:4, :], c_st[d])
                tg = spool.tile([128, 2, BC], BF16, tag=f"tg{d}",
                                name=f"tg{d}")
                # tanh(g) = 2*sigmoid(2g) - 1 (g-rows prescaled by 2)
                nc.vector.scalar_tensor_tensor(
                    tg[:], g[:, 6:8, :], 2.0, onesd[:],
                    op0=ALU.mult, op1=ALU.subtract)
                ig = spool.tile([128, 2, BC], BF16, tag=f"ig{d}",
                                name=f"ig{d}")
                nc.vector.tensor_mul(ig[:], tg[:], g[:, 0:2, :])
                cn = spool.tile([128, 2, BC], F32, tag=f"c{d}", name=f"c{d}")
                nc.vector.tensor_add(cn[:], fc[:], ig[:])
                return cn

            def tanh_phase(d, cn):
                th = spool.tile([128, 2, BC], BF16, tag=f"th{d}",
                                name=f"th{d}")
                nc.scalar.activation(th[:], cn[:], AF.Tanh)
                return th

            def hm_phase(d, t, g, cn, th):
                col = (t if d == 0 else S - 1 - t) * BC
                nc.vector.tensor_mul(hts[d][:, :, col:col + BC],
                                     g[:, 4:6, :], th[:])
                c_st[d] = cn[:]

            def new_z(d):
                return zps_pool.tile([128, 8, BC], F32, tag=f"z{d}",
                                     name=f"z{d}")

            # backward chain runs D slots behind the forward chain so its
            # Act/DVE ops always have slot-old inputs and never stall the
            # forward chain through the in-order engine queues
            D = 2
            zcur = {0: new_z(0)}
            emit_bias_x(zcur[0], 0, 0)
            for slot in range(S + D):
                fon = slot < S
                bon = slot >= D
                tb_ = slot - D
                if bon:
                    emit_h(zcur[1], 1, tb_)
                if fon:
                    emit_h(zcur[0], 0, slot)
                gf = sig_phase(0, zcur[0]) if fon else None
                gb = sig_phase(1, zcur[1]) if bon else None
                cf = dve_phase(0, gf) if fon else None
                cb = dve_phase(1, gb) if bon else None
                tf = tanh_phase(0, cf) if fon else None
                tbh = tanh_phase(1, cb) if bon else None
                if fon:
                    hm_phase(0, slot, gf, cf, tf)
                if bon:
                    hm_phase(1, tb_, gb, cb, tbh)
                if slot < S - 1:
                    zf = new_z(0)
                    emit_bias_x(zf, 0, slot + 1)
                    zcur[0] = zf
                if 0 <= slot - D + 1 < S:
                    zb = new_z(1)
                    emit_bias_x(zb, 1, slot - D + 1)
                    zcur[1] = zb
                for b in em_ready.get(slot, []):
                    emit_emission(b, slot >= S - 1)

            # ---- CRF: meet-in-the-middle forward/backward ---------------
            # emitted in bursts of 4 steps per chain to amortize the
            # cross-chain head-of-line coupling on the in-order engines
            half = S // 2  # alpha covers em 0..63, gamma covers 127..64
            a_prev = em_sb[:, 0:BC]
            gma = gpool.tile([TA, BC], BF16, tag="gma", name="gma")
            nc.vector.tensor_scalar_mul(
                gma[:], em_sb[:, (S - 1) * BC:S * BC], eend_sb[:])
            g_prev = gma[:]

            def alpha_step(i):
                nonlocal a_prev
                ta_ = i + 1
                aps = crfps_pool.tile([TA, BC], F32, tag="crf")
                nc.tensor.matmul(aps[:], mp_sb[:], a_prev,
                                 start=True, stop=True)
                a_new = gpool.tile([TA, BC], BF16, tag="a", name="a")
                nc.vector.tensor_mul(
                    a_new[:], aps[:], em_sb[:, ta_ * BC:(ta_ + 1) * BC])
                a_prev = a_new[:]

            def gamma_step(i):
                nonlocal g_prev
                tb_ = S - 2 - i
                gps = crfps_pool.tile([TA, BC], F32, tag="crf")
                nc.tensor.matmul(gps[:], mpt_sb[:], g_prev,
                                 start=True, stop=True)
                g_new = gpool.tile([TA, BC], BF16, tag="gma", name="gma")
                nc.vector.tensor_mul(
                    g_new[:], gps[:], em_sb[:, tb_ * BC:(tb_ + 1) * BC])
                g_prev = g_new[:]

            for i in range(half - 1):
                alpha_step(i)
                gamma_step(i)

            # Z = alpha_63 . (M gamma_64)
            wps = crfps_pool.tile([TA, BC], F32, tag="crf")
            nc.tensor.matmul(wps[:], mpt_sb[:], g_prev, start=True, stop=True)
            u = wpool.tile([TA, BC], F32, tag="u")
            nc.vector.tensor_mul(u[:], wps[:], a_prev)
            zsc = crfps_pool.tile([1, BC + 8], F32, tag="zsc", bufs=1)
            nc.tensor.matmul(zsc[:, 0:BC], ones77[:], u[:],
                             start=True, stop=True)
            logs = wpool.tile([1, BC], F32, tag="logs")
            nc.scalar.activation(logs[:], zsc[:, 0:BC], AF.Ln)
            logsum = wpool.tile([1, 1], F32, tag="logsum")
            nc.vector.tensor_reduce(logsum[:], logs[:],
                                    axis=mybir.AxisListType.X, op=ALU.add)

            # ---- gold emission score sum --------------------------------
            tot = wpool.tile([T, 1], F32, tag="tot")
            nc.vector.tensor_add(tot[:], em_accs[0][:], em_accs[1][:])
            for acc in em_accs[2:]:
                nc.vector.tensor_add(tot[:], tot[:], acc[:])
            nc.tensor.matmul(zsc[:, BC:BC + 1], ones77[0:T, :], tot[:],
                             start=True, stop=True)

            res = wpool.tile([1, 2], F32, tag="res")
            nc.vector.tensor_copy(res[:, 0:1], logsum[:])
            nc.vector.tensor_copy(res[:, 1:2], zsc[:, BC:BC + 1])
            nc.sync.dma_start(out_d[:], res[:])

    return nc


# ---------------------------------------------------------------- host side
def _gate_perm():
    """PyTorch gate order i,f,g,o -> reordered i,f,o,g (rows of W/b)."""
    return np.concatenate([
        np.arange(0, HD),            # i
        np.arange(HD, 2 * HD),       # f
        np.arange(3 * HD, 4 * HD),   # o
        np.arange(2 * HD, 3 * HD),   # g
    ])


def _pack_w_t(w, perm, nchunks, gscale):
    """w: [G4, kdim] -> [128, nchunks, G4] bf16 with
    out[p, c, g] = w[perm[g], c*128+p] * gscale[g]."""
    wp = np.asarray(w, dtype=np.float32)[perm, :] * gscale[:, None]
    out = np.empty((128, nchunks, G4), dtype=ml_dtypes.bfloat16)
    for c in range(nchunks):
        out[:, c, :] = wp[:, c * 128:(c + 1) * 128].T.astype(ml_dtypes.bfloat16)
    return out


def prep_inputs(inputs):
    """Build per-core input maps + host constants."""
    ids = np.asarray(inputs["input_ids"])
    tags = np.asarray(inputs["tag_ids"])
    lengths = np.asarray(inputs["lengths"])
    perm = _gate_perm()
    # gate g (index 768:1024 after perm) prescaled by 2 for the
    # tanh(x) = 2*sigmoid(2x)-1 identity
    gscale = np.ones(G4, dtype=np.float32)
    gscale[3 * HD:] = 2.0

    embed_bf = np.asarray(inputs["embed_table"]).astype(ml_dtypes.bfloat16)

    def gather_xt(flat_ids):
        g = embed_bf[flat_ids]                       # [NTOK, E] bf16
        return np.ascontiguousarray(
            g.reshape(NTOK, 4, 128).transpose(2, 1, 0))

    wih_pack = np.stack([_pack_w_t(inputs["W_ih_f"], perm, 4, gscale),
                         _pack_w_t(inputs["W_ih_b"], perm, 4, gscale)],
                        axis=1)                      # [128, 2, 4, G4]
    whh_pack = np.stack([_pack_w_t(inputs["W_hh_f"], perm, 2, gscale),
                         _pack_w_t(inputs["W_hh_b"], perm, 2, gscale)],
                        axis=1)                      # [128, 2, 2, G4]
    bias_f = ((np.asarray(inputs["b_ih_f"]) + np.asarray(inputs["b_hh_f"]))
              [perm] * gscale)
    bias_b = ((np.asarray(inputs["b_ih_b"]) + np.asarray(inputs["b_hh_b"]))
              [perm] * gscale)
    bias_pack = np.stack([bias_f.reshape(8, 128), bias_b.reshape(8, 128)]
                         )[None].astype(ml_dtypes.bfloat16)  # [1, 2, 8, 128]

    wo = np.asarray(inputs["W_out"])                 # [T, H]
    wout_pack = np.empty((128, 2, 2, T), dtype=ml_dtypes.bfloat16)
    for d in range(2):
        for k in range(2):
            sl = slice(d * 256 + k * 128, d * 256 + (k + 1) * 128)
            wout_pack[:, k, d, :] = wo[:, sl].T.astype(ml_dtypes.bfloat16)

    trans = np.asarray(inputs["trans"]).astype(np.float64)
    start_t = np.asarray(inputs["start_trans"]).astype(np.float64)
    end_t = np.asarray(inputs["end_trans"]).astype(np.float64)
    bout = np.asarray(inputs["b_out"]).astype(np.float64)
    kappa = float(np.log(np.exp(trans).sum(axis=0).mean()))

    mp = np.zeros((TA, TA), dtype=np.float64)
    mp[0:T, 0:T] = np.exp(trans - kappa)
    mp[0:T, T] = np.exp(end_t - kappa)
    mp[T, T] = 1.0
    eend = np.zeros((TA, 1), dtype=np.float32)
    eend[0:T, 0] = np.exp(end_t)
    eend[T, 0] = 1.0
    bvec = np.zeros((T, 2), dtype=np.float32)
    bvec[:, 0] = bout
    bvec[:, 1] = bout + start_t

    h0 = np.asarray(inputs["h0"])                    # [2, B, HD]
    c0 = np.asarray(inputs["c0"])

    in_maps = []
    k_len_total = 0
    gold_host_total = 0.0
    for c in range(N_CORES):
        bs = slice(c * BC, (c + 1) * BC)
        ids_c = ids[bs]
        tags_c = tags[bs]
        len_c = lengths[bs].astype(np.int64)
        k_len_total += int(np.minimum(len_c, S - 1).sum())

        idx_f = ids_c.T.reshape(-1)                    # token (s, b) order
        idx_b = ids_c[:, ::-1].T.reshape(-1)
        xt = np.stack([gather_xt(idx_f), gather_xt(idx_b)])

        svec = np.arange(S)[None, :]
        valid = (svec < len_c[:, None]).T.reshape(-1)  # [(s, b)]
        ohm = np.zeros((T, NTOK), dtype=ml_dtypes.bfloat16)
        tt = tags_c.T.reshape(-1)
        pos = np.arange(NTOK)
        ohm[tt[valid], pos[valid]] = 1
        vm = np.broadcast_to(valid.astype(ml_dtypes.bfloat16),
                             (T, NTOK)).copy()
        padr = (~valid).astype(ml_dtypes.bfloat16)[None, :]

        # gold-path table part (trans/start/end/b_out counts) on host
        gh = 0.0
        for b in range(BC):
            L = int(len_c[b])
            tg = tags_c[b, :L]
            gh += float(trans[tg[:-1], tg[1:]].sum())
            gh += float(start_t[tg[0]] + end_t[tg[-1]])
            gh += float(bout[tg].sum())
        gold_host_total += gh

        h0t = np.zeros((128, 2, 2, BC), dtype=ml_dtypes.bfloat16)
        c0t = np.zeros((128, 2, 2, BC), dtype=np.float32)
        for d in range(2):
            for k in range(2):
                h0t[:, d, k, :] = h0[d][bs][:, k * 128:(k + 1) * 128].T
                c0t[:, d, k, :] = c0[d][bs][:, k * 128:(k + 1) * 128].T

        in_maps.append(dict(
            xt=xt, wih=wih_pack, whh=whh_pack, biast=bias_pack,
            wout=wout_pack, h0t=h0t, c0t=c0t,
            mp=mp.astype(ml_dtypes.bfloat16),
            mpt=mp.T.copy().astype(ml_dtypes.bfloat16),
            eend=eend, bvec=bvec, ohm=ohm, vmask=vm, padrow=padr,
        ))

    return in_maps, dict(kappa=kappa, k_len_total=k_len_total,
                         gold_host_total=gold_host_total)


def finalize(results, host):
    logz = sum(float(r["out"][0, 0]) for r in results)
    gold_em = sum(float(r["out"][0, 1]) for r in results)
    logz += host["kappa"] * host["k_len_total"]
    score = gold_em + host["gold_host_total"]
    return np.float32((logz - score) / B)


# ---------------------------------------------------------------- entry point
_COMPILED = {}


def kernel(**inputs):
    """Full-input BiLSTM-CRF loss on 8 NeuronCores (data parallel)."""
    from concourse.bass_utils import run_bass_kernel_spmd
    in_maps, host = prep_inputs(inputs)
    if "nc" not in _COMPILED:
        _COMPILED["nc"] = build_nc()
    nc = _COMPILED["nc"]
    res = run_bass_kernel_spmd(nc, in_maps, core_ids=list(range(N_CORES)))
    return np.asarray(finalize(res.results, host))


# revision 38
# speedup vs baseline: 1.0133x; 1.0133x over previous
"""BiLSTM-CRF loss kernel for Trainium2, 8-core data parallel.

Per-core (batch shard of 32), feature-major ("transposed") layout throughout:
gates/features live on partitions, batch on the free dim, so every elementwise
op runs at 128-partition occupancy with a small free size.

  - z_t for each direction accumulates in PSUM as [128 gates-in-chunk,
    8 chunks, batch]: per (chunk, dir) group = 1 bias matmul (K=1 ones rhs)
    + 4 x-projection matmuls (xg in [E, token] layout, consumed in-loop; no
    DRAM z roundtrip) + 2 recurrent matmuls off the transposed h buffer.
  - one sigmoid covers all 8 gate chunks; the g-gate rows of W/b are
    host-prescaled by 2 so tanh(g) = 2*sigmoid(z_g) - 1, done as a single
    DVE scalar_tensor_tensor. f*c runs on GpSimd (Pool) off the DVE path.
  - h = sigma_o * tanh(c) is written directly into the persistent transposed
    h buffer [128, k, dir, token] feeding both the next step's matmuls and
    the emission matmuls -- no PE transposes anywhere.
  - emissions (em = Wout.[hf;hb]) are computed per 512-token block as soon
    as both chains have covered it, with exp/vmask/gold-dot fused in.
  - CRF partition function in scaled linear space with an absorbing 77th
    tag: meet-in-the-middle (alpha forward 64 steps, beta/gamma backward 64
    steps, run concurrently), Z = alpha_63 . (M gamma_64).
Host combines the 8 per-core partial sums into the scalar loss.
"""

import numpy as np
import ml_dtypes

import concourse.bass as bass
import concourse.mybir as mybir
from concourse.tile import TileContext
from concourse.vector_clock import ScopedClock
from concourse.alu_op_type import AluOpType as ALU

N_CORES = 8
B, S, E, HD, T, V = 256, 128, 512, 256, 76, 30000
BC = B // N_CORES          # 32 batch per core
G4 = 4 * HD                # 1024 gates per direction
TA = T + 1                 # 77 tags with absorber
NTOK = S * BC              # 4096 tokens per direction per core

dt = mybir.dt
F32, BF16 = dt.float32, dt.bfloat16
AF = mybir.ActivationFunctionType

# ---------------------------------------------------------------- tile patch
# This walrus build rejects >1 sem wait on CTRL-class (Drain/NoOp)
# instructions; split the Tile tail-drain waits across preceding NOPs.
_MAX_WAITS = 1
_WAIT_LIMITS = {}


def _split_excess_waits(nc):
    """Non-DMA instructions accept only one sem wait on this walrus build;
    move excess waits onto NOPs spliced in front (same engine, same order)."""
    for f in nc.m.functions:
        stack = list(f.blocks)
        while stack:
            bb = stack.pop()
            for sub in getattr(bb, "blocks", []) or []:
                stack.append(sub)
            insts = getattr(bb, "instructions", None)
            if not insts:
                continue
            newlist = []
            changed = False
            for inst in insts:
                si = inst.sync_info
                lim = _WAIT_LIMITS.get(type(inst).__name__, 1)
                if si is not None and si.on_wait and len(si.on_wait) > lim:
                    waits = list(si.on_wait)
                    si.on_wait = waits[-lim:]
                    for w in waits[:-lim]:
                        nop = mybir.InstNoOp(
                            name=f"I-wsplit{nc.next_id()}", ins=[], outs=[],
                            engine=inst.engine,
                            sync_info=mybir.SyncInfo(on_wait=[w], on_update=[]),
                        )
                        newlist.append(nop)
                    changed = True
                newlist.append(inst)
            if changed:
                insts[:] = newlist


def _patched_drain_and_barrier(self, tick_clock, wait_clock):
    nc = self.nc
    _split_excess_waits(nc)
    nops = [nc.sync.nop(nofuse=True, hint=f"waitsplit{i}") for i in range(16)]
    drain_inst = nc.sync.drain()
    wait_clock.add_sem_waits(
        drain_inst.ins, ScopedClock({None: tick_clock.global_clock})
    )
    si = drain_inst.ins.sync_info
    if si is not None and si.on_wait and len(si.on_wait) > _MAX_WAITS:
        waits = list(si.on_wait)
        chunks = [waits[i:i + _MAX_WAITS] for i in range(0, len(waits), _MAX_WAITS)]
        si.on_wait = chunks[-1]
        assert len(chunks) - 1 <= len(nops), "too many wait chunks"
        for i, ch in enumerate(chunks[:-1]):
            ni = nops[i].ins
            if ni.sync_info is None:
                ni.sync_info = mybir.SyncInfo(on_wait=ch, on_update=[])
            else:
                ni.sync_info.on_wait = list(ni.sync_info.on_wait) + ch
    nc.all_engine_barrier()
    assert self.sems is not None
    popped = nc._tile_sem_poison_stack.pop()
    assert popped is self._sem_poison
    allsems = list(self.sems.allocated().values())
    for i in range(0, len(allsems), 8):
        nc.clear_and_free_semaphores(allsems[i:i + 8])
    nc.all_engine_barrier()


def apply_tile_patch():
    TileContext._drain_and_barrier = _patched_drain_and_barrier


# ---------------------------------------------------------------- builder
def build_nc():
    apply_tile_patch()
    nc = bass.Bass("TRN2", target_bir_lowering=False, debug=False,
                   num_devices=N_CORES)

    xt_d = nc.dram_tensor("xt", [2, 128, 4, NTOK], BF16, kind="ExternalInput")
    wih_d = nc.dram_tensor("wih", [128, 2, 4, G4], BF16, kind="ExternalInput")
    whh_d = nc.dram_tensor("whh", [128, 2, 2, G4], BF16, kind="ExternalInput")
    bias_d = nc.dram_tensor("biast", [1, 2, 8, 128], BF16, kind="ExternalInput")
    wout_d = nc.dram_tensor("wout", [128, 2, 2, T], BF16, kind="ExternalInput")
    h0t_d = nc.dram_tensor("h0t", [128, 2, 2, BC], BF16, kind="ExternalInput")
    c0t_d = nc.dram_tensor("c0t", [128, 2, 2, BC], F32, kind="ExternalInput")
    mp_d = nc.dram_tensor("mp", [TA, TA], BF16, kind="ExternalInput")
    mpt_d = nc.dram_tensor("mpt", [TA, TA], BF16, kind="ExternalInput")
    eend_d = nc.dram_tensor("eend", [TA, 1], F32, kind="ExternalInput")
    bvec_d = nc.dram_tensor("bvec", [T, 2], F32, kind="ExternalInput")
    ohm_d = nc.dram_tensor("ohm", [T, NTOK], BF16, kind="ExternalInput")
    vmask_d = nc.dram_tensor("vmask", [T, NTOK], BF16, kind="ExternalInput")
    padrow_d = nc.dram_tensor("padrow", [1, NTOK], BF16, kind="ExternalInput")
    out_d = nc.dram_tensor("out", [1, 2], F32, kind="ExternalOutput")

    NB = S // 16  # 8 emission blocks of 512 tokens
    # slot (0-based) after which emission block b is fully available; the
    # backward chain is software-pipelined two slots behind the forward one
    em_ready = {}
    for b in range(NB):
        r = max(16 * b + 15, S - 1 - 16 * b + 2)
        em_ready.setdefault(r, []).append(b)

    with TileContext(nc) as tc:
        with (
            tc.tile_pool(name="const", bufs=1) as cpool,
            tc.tile_pool(name="hbuf", bufs=1) as hpool,
            tc.tile_pool(name="gate", bufs=3) as gpool,
            tc.tile_pool(name="cell", bufs=3) as spool,
            tc.tile_pool(name="work", bufs=3) as wpool,
            tc.tile_pool(name="zps", bufs=2, space="PSUM") as zps_pool,
            tc.tile_pool(name="emps", bufs=1, space="PSUM") as emps_pool,
            tc.tile_pool(name="crfps", bufs=2, space="PSUM") as crfps_pool,
        ):
            # ---- constants / weights into SBUF.  DMA order is the startup
            # critical path: everything step 0 needs (bias, h0, c0, wih,
            # whh, first xg chunk) goes first; the rest streams in behind.
            bias_sb = cpool.tile([1, 2, 8, 128], BF16)
            nc.sync.dma_start(bias_sb[:], bias_d[:])
            h0t_sb = cpool.tile([128, 2, 2, BC], BF16)
            nc.sync.dma_start(h0t_sb[:], h0t_d[:])
            c0t_sb = cpool.tile([128, 2, 2, BC], F32)
            nc.sync.dma_start(c0t_sb[:], c0t_d[:])
            wih_sb = cpool.tile([128, 2, 4, G4], BF16)
            nc.sync.dma_start(wih_sb[:, 0], wih_d.ap()[:, 0])
            whh_sb = cpool.tile([128, 2, 2, G4], BF16)
            nc.sync.dma_start(whh_sb[:], whh_d[:])
            xg = {d: hpool.tile([128, 4, NTOK], BF16, name=f"xg{d}")
                  for d in range(2)}
            NCH = 4
            CW = NTOK // NCH
            nc.sync.dma_start(xg[0][:, :, 0:CW], xt_d.ap()[0, :, :, 0:CW])
            nc.sync.dma_start(wih_sb[:, 1], wih_d.ap()[:, 1])
            nc.sync.dma_start(xg[1][:, :, 0:CW], xt_d.ap()[1, :, :, 0:CW])

            wout_sb = cpool.tile([128, 2, 2, T], BF16)
            nc.sync.dma_start(wout_sb[:], wout_d[:])
            mp_sb = cpool.tile([TA, TA], BF16)
            nc.sync.dma_start(mp_sb[:], mp_d[:])
            mpt_sb = cpool.tile([TA, TA], BF16)
            nc.sync.dma_start(mpt_sb[:], mpt_d[:])
            eend_sb = cpool.tile([TA, 1], F32)
            nc.sync.dma_start(eend_sb[:], eend_d[:])
            bvec_sb = cpool.tile([T, 2], F32)
            nc.sync.dma_start(bvec_sb[:], bvec_d[:])

            ones1 = cpool.tile([1, BC], BF16)
            nc.vector.memset(ones1[:], 1.0)
            onesd = cpool.tile([128, 2, BC], BF16)
            nc.vector.memset(onesd[:], 1.0)
            ones77 = cpool.tile([TA, 1], F32)
            nc.vector.memset(ones77[:], 1.0)

            # remaining xg chunks stream in behind the first ones
            for c in range(1, NCH):
                for d in range(2):
                    nc.sync.dma_start(
                        xg[d][:, :, c * CW:(c + 1) * CW],
                        xt_d.ap()[d, :, :, c * CW:(c + 1) * CW])
            # transposed h, one tile per direction: [128, k-chunk, token]
            hts = {d: hpool.tile([128, 2, NTOK], BF16, name=f"hts{d}")
                   for d in range(2)}
            # emissions (scaled-exp'd), bf16, absorber row 76
            em_sb = hpool.tile([TA, NTOK], BF16, name="em")
            ohm_sb = hpool.tile([T, NTOK], BF16, name="ohm")
            nc.sync.dma_start(ohm_sb[:], ohm_d[:])
            vm_sb = hpool.tile([T, NTOK], BF16, name="vm")
            nc.sync.dma_start(vm_sb[:], vmask_d[:])
            nc.sync.dma_start(em_sb[T:TA, :], padrow_d[:])

            # ---- z PSUM tile helpers -----------------------------------
            def emit_bias_x(zt, d, t):
                """bias + x-projection matmuls of direction d for step t into
                PSUM tile zt [128, 8 gate-chunk, BC]."""
                tok = slice(t * BC, (t + 1) * BC)
                for gc in range(8):
                    nc.tensor.matmul(
                        zt[:, gc, :], bias_sb[:, d, gc, :],
                        ones1[:], start=True, stop=False)
                for ek in range(4):
                    for gc in range(8):
                        nc.tensor.matmul(
                            zt[:, gc, :],
                            wih_sb[:, d, ek, gc * 128:(gc + 1) * 128],
                            xg[d][:, ek, tok], start=False, stop=False)

            def emit_h(zt, d, t):
                """recurrent matmuls (Whh . h_{t-1}) closing step t's groups."""
                for k in range(2):
                    if t == 0:
                        hk = h0t_sb[:, d, k, :]
                    else:
                        col = (t - 1 if d == 0 else S - t) * BC
                        hk = hts[d][:, k, col:col + BC]
                    for gc in range(8):
                        nc.tensor.matmul(
                            zt[:, gc, :],
                            whh_sb[:, d, k, gc * 128:(gc + 1) * 128],
                            hk, start=False, stop=(k == 1))

            # ---- emission block -----------------------------------------
            em_accs = []
            deferred_red = []

            def emit_emission(b, late):
                blk = slice(b * 512, (b + 1) * 512)
                ps = emps_pool.tile([T, 512], F32, tag="emps")
                i = 0
                for d in range(2):
                    for k in range(2):
                        nc.tensor.matmul(ps[:], wout_sb[:, k, d, :],
                                         hts[d][:, k, blk],
                                         start=(i == 0), stop=(i == 3))
                        i += 1
                # gold-path dot on raw em (b_out part handled on host)
                scr = wpool.tile([T, 512], BF16, tag=f"scr{b}", bufs=1,
                                 name=f"scr{b}")
                nc.vector.tensor_mul(scr[:], ps[:], ohm_sb[:, blk])
                acc = wpool.tile([T, 1], F32, tag=f"emacc{b}", bufs=1,
                                 name=f"emacc{b}")
                em_accs.append(acc)
                nc.vector.tensor_reduce(acc[:], scr[:],
                                        axis=mybir.AxisListType.X,
                                        op=ALU.add)
                # scaled emissions: exp(em + b_out [+ start on col 0])
                if b == 0:
                    nc.scalar.activation(em_sb[0:T, 0:BC], ps[:, 0:BC],
                                         AF.Exp, bias=bvec_sb[:, 1:2])
                    nc.scalar.activation(em_sb[0:T, BC:512], ps[:, BC:512],
                                         AF.Exp, bias=bvec_sb[:, 0:1])
                else:
                    nc.scalar.activation(em_sb[0:T, blk], ps[:],
                                         AF.Exp, bias=bvec_sb[:, 0:1])
                nc.vector.tensor_mul(em_sb[0:T, blk], em_sb[0:T, blk],
                                     vm_sb[:, blk])

            # ---- LSTM loop ----------------------------------------------
            # Forward chain runs in slot t = its step t; the backward chain
            # is software-pipelined one slot behind (step t in slot t+1) so
            # its Act/DVE ops always have ready inputs and can never stall
            # the forward chain through the in-order engine queues.
            c_st = {d: c0t_sb[:, d, :, :] for d in range(2)}

            def sig_phase(d, zt):
                g = gpool.tile([128, 8, BC], BF16, tag=f"g{d}", name=f"g{d}")
                nc.scalar.activation(g[:], zt[:], AF.Sigmoid)
                return g

            def dve_phase(d, g):
                fc = spool.tile([128, 2, BC], F32, tag=f"fc{d}",
                                name=f"fc{d}")
                nc.gpsimd.tensor_mul(fc[:], g[:, 2:4, :], c_st[d])
                tg = spool.tile([128, 2, BC], BF16, tag=f"tg{d}",
                                name=f"tg{d}")
                # tanh(g) = 2*sigmoid(2g) - 1 (g-rows prescaled by 2)
                nc.vector.scalar_tensor_tensor(
                    tg[:], g[:, 6:8, :], 2.0, onesd[:],
                    op0=ALU.mult, op1=ALU.subtract)
                ig = spool.tile([128, 2, BC], BF16, tag=f"ig{d}",
                                name=f"ig{d}")
                nc.vector.tensor_mul(ig[:], tg[:], g[:, 0:2, :])
                cn = spool.tile([128, 2, BC], F32, tag=f"c{d}", name=f"c{d}")
                nc.vector.tensor_add(cn[:], fc[:], ig[:])
                return cn

            def tanh_phase(d, cn):
                th = spool.tile([128, 2, BC], BF16, tag=f"th{d}",
                                name=f"th{d}")
                nc.scalar.activation(th[:], cn[:], AF.Tanh)
                return th

            def hm_phase(d, t, g, cn, th):
                col = (t if d == 0 else S - 1 - t) * BC
                nc.vector.tensor_mul(hts[d][:, :, col:col + BC],
                                     g[:, 4:6, :], th[:])
                c_st[d] = cn[:]

            def new_z(d):
                return zps_pool.tile([128, 8, BC], F32, tag=f"z{d}",
                                     name=f"z{d}")

            # backward chain runs D slots behind the forward chain so its
            # Act/DVE ops always have slot-old inputs and never stall the
            # forward chain through the in-order engine queues
            D = 2
            zcur = {0: new_z(0)}
            emit_bias_x(zcur[0], 0, 0)
            for slot in range(S + D):
                fon = slot < S
                bon = slot >= D
                tb_ = slot - D
                if bon:
                    emit_h(zcur[1], 1, tb_)
                if fon:
                    emit_h(zcur[0], 0, slot)
                gf = sig_phase(0, zcur[0]) if fon else None
                gb = sig_phase(1, zcur[1]) if bon else None
                cf = dve_phase(0, gf) if fon else None
                cb = dve_phase(1, gb) if bon else None
                tf = tanh_phase(0, cf) if fon else None
                tbh = tanh_phase(1, cb) if bon else None
                if fon:
                    hm_phase(0, slot, gf, cf, tf)
                if bon:
                    hm_phase(1, tb_, gb, cb, tbh)
                if slot < S - 1:
                    zf = new_z(0)
                    emit_bias_x(zf, 0, slot + 1)
                    zcur[0] = zf
                if 0 <= slot - D + 1 < S:
                    zb = new_z(1)
                    emit_bias_x(zb, 1, slot - D + 1)
                    zcur[1] = zb
                for b in em_ready.get(slot, []):
                    emit_emission(b, slot >= S - 1)

            # ---- CRF: meet-in-the-middle forward/backward ---------------
            # emitted in bursts of 4 steps per chain to amortize the
            # cross-chain head-of-line coupling on the in-order engines
            half = S // 2  # alpha covers em 0..63, gamma covers 127..64
            a_prev = em_sb[:, 0:BC]
            gma = gpool.tile([TA, BC], BF16, tag="gma", name="gma")
            nc.vector.tensor_scalar_mul(
                gma[:], em_sb[:, (S - 1) * BC:S * BC], eend_sb[:])
            g_prev = gma[:]

            def alpha_step(i):
                nonlocal a_prev
                ta_ = i + 1
                aps = crfps_pool.tile([TA, BC], F32, tag="crf")
                nc.tensor.matmul(aps[:], mp_sb[:], a_prev,
                                 start=True, stop=True)
                a_new = gpool.tile([TA, BC], BF16, tag="a", name="a")
                nc.vector.tensor_mul(
                    a_new[:], aps[:], em_sb[:, ta_ * BC:(ta_ + 1) * BC])
                a_prev = a_new[:]

            def gamma_step(i):
                nonlocal g_prev
                tb_ = S - 2 - i
                gps = crfps_pool.tile([TA, BC], F32, tag="crf")
                nc.tensor.matmul(gps[:], mpt_sb[:], g_prev,
                                 start=True, stop=True)
                g_new = gpool.tile([TA, BC], BF16, tag="gma", name="gma")
                nc.vector.tensor_mul(
                    g_new[:], gps[:], em_sb[:, tb_ * BC:(tb_ + 1) * BC])
                g_prev = g_new[:]

            for i in range(half - 1):
                alpha_step(i)
                gamma_step(i)

            # Z = alpha_63 . (M gamma_64)
            wps = crfps_pool.tile([TA, BC], F32, tag="crf")
            nc.tensor.matmul(wps[:], mpt_sb[:], g_prev, start=True, stop=True)
            u = wpool.tile([TA, BC], F32, tag="u")
            nc.vector.tensor_mul(u[:], wps[:], a_prev)
            zsc = crfps_pool.tile([1, BC + 8], F32, tag="zsc", bufs=1)
            nc.tensor.matmul(zsc[:, 0:BC], ones77[:], u[:],
                             start=True, stop=True)
            logs = wpool.tile([1, BC], F32, tag="logs")
            nc.scalar.activation(logs[:], zsc[:, 0:BC], AF.Ln)
            logsum = wpool.tile([1, 1], F32, tag="logsum")
            nc.vector.tensor_reduce(logsum[:], logs[:],
                                    axis=mybir.AxisListType.X, op=ALU.add)

            # ---- gold emission score sum --------------------------------
            tot = wpool.tile([T, 1], F32, tag="tot")
            nc.vector.tensor_add(tot[:], em_accs[0][:], em_accs[1][:])
            for acc in em_accs[2:]:
                nc.vector.tensor_add(tot[:], tot[:], acc[:])
            nc.tensor.matmul(zsc[:, BC:BC + 1], ones77[0:T, :], tot[:],
                             start=True, stop=True)

            res = wpool.tile([1, 2], F32, tag="res")
            nc.vector.tensor_copy(res[:, 0:1], logsum[:])
            nc.vector.tensor_copy(res[:, 1:2], zsc[:, BC:BC + 1])
            nc.sync.dma_start(out_d[:], res[:])

    return nc


# ---------------------------------------------------------------- host side
def _gate_perm():
    """PyTorch gate order i,f,g,o -> reordered i,f,o,g (rows of W/b)."""
    return np.concatenate([
        np.arange(0, HD),            # i
        np.arange(HD, 2 * HD),       # f
        np.arange(3 * HD, 4 * HD),   # o
        np.arange(2 * HD, 3 * HD),   # g
    ])


def _pack_w_t(w, perm, nchunks, gscale):
    """w: [G4, kdim] -> [128, nchunks, G4] bf16 with
    out[p, c, g] = w[perm[g], c*128+p] * gscale[g]."""
    wp = np.asarray(w, dtype=np.float32)[perm, :] * gscale[:, None]
    out = np.empty((128, nchunks, G4), dtype=ml_dtypes.bfloat16)
    for c in range(nchunks):
        out[:, c, :] = wp[:, c * 128:(c + 1) * 128].T.astype(ml_dtypes.bfloat16)
    return out


def prep_inputs(inputs):
    """Build per-core input maps + host constants."""
    ids = np.asarray(inputs["input_ids"])
    tags = np.asarray(inputs["tag_ids"])
    lengths = np.asarray(inputs["lengths"])
    perm = _gate_perm()
    # gate g (index 768:1024 after perm) prescaled by 2 for the
    # tanh(x) = 2*sigmoid(2x)-1 identity
    gscale = np.ones(G4, dtype=np.float32)
    gscale[3 * HD:] = 2.0

    embed_bf = np.asarray(inputs["embed_table"]).astype(ml_dtypes.bfloat16)

    def gather_xt(flat_ids):
        g = embed_bf[flat_ids]                       # [NTOK, E] bf16
        return np.ascontiguousarray(
            g.reshape(NTOK, 4, 128).transpose(2, 1, 0))

    wih_pack = np.stack([_pack_w_t(inputs["W_ih_f"], perm, 4, gscale),
                         _pack_w_t(inputs["W_ih_b"], perm, 4, gscale)],
                        axis=1)                      # [128, 2, 4, G4]
    whh_pack = np.stack([_pack_w_t(inputs["W_hh_f"], perm, 2, gscale),
                         _pack_w_t(inputs["W_hh_b"], perm, 2, gscale)],
                        axis=1)                      # [128, 2, 2, G4]
    bias_f = ((np.asarray(inputs["b_ih_f"]) + np.asarray(inputs["b_hh_f"]))
              [perm] * gscale)
    bias_b = ((np.asarray(inputs["b_ih_b"]) + np.asarray(inputs["b_hh_b"]))
              [perm] * gscale)
    bias_pack = np.stack([bias_f.reshape(8, 128), bias_b.reshape(8, 128)]
                         )[None].astype(ml_dtypes.bfloat16)  # [1, 2, 8, 128]

    wo = np.asarray(inputs["W_out"])                 # [T, H]
    wout_pack = np.empty((128, 2, 2, T), dtype=ml_dtypes.bfloat16)
    for d in range(2):
        for k in range(2):
            sl = slice(d * 256 + k * 128, d * 256 + (k + 1) * 128)
            wout_pack[:, k, d, :] = wo[:, sl].T.astype(ml_dtypes.bfloat16)

    trans = np.asarray(inputs["trans"]).astype(np.float64)
    start_t = np.asarray(inputs["start_trans"]).astype(np.float64)
    end_t = np.asarray(inputs["end_trans"]).astype(np.float64)
    bout = np.asarray(inputs["b_out"]).astype(np.float64)
    kappa = float(np.log(np.exp(trans).sum(axis=0).mean()))

    mp = np.zeros((TA, TA), dtype=np.float64)
    mp[0:T, 0:T] = np.exp(trans - kappa)
    mp[0:T, T] = np.exp(end_t - kappa)
    mp[T, T] = 1.0
    eend = np.zeros((TA, 1), dtype=np.float32)
    eend[0:T, 0] = np.exp(end_t)
    eend[T, 0] = 1.0
    bvec = np.zeros((T, 2), dtype=np.float32)
    bvec[:, 0] = bout
    bvec[:, 1] = bout + start_t

    h0 = np.asarray(inputs["h0"])                    # [2, B, HD]
    c0 = np.asarray(inputs["c0"])

    in_maps = []
    k_len_total = 0
    gold_host_total = 0.0
    for c in range(N_CORES):
        bs = slice(c * BC, (c + 1) * BC)
        ids_c = ids[bs]
        tags_c = tags[bs]
        len_c = lengths[bs].astype(np.int64)
        k_len_total += int(np.minimum(len_c, S - 1).sum())

        idx_f = ids_c.T.reshape(-1)                    # token (s, b) order
        idx_b = ids_c[:, ::-1].T.reshape(-1)
        xt = np.stack([gather_xt(idx_f), gather_xt(idx_b)])

        svec = np.arange(S)[None, :]
        valid = (svec < len_c[:, None]).T.reshape(-1)  # [(s, b)]
        ohm = np.zeros((T, NTOK), dtype=ml_dtypes.bfloat16)
        tt = tags_c.T.reshape(-1)
        pos = np.arange(NTOK)
        ohm[tt[valid], pos[valid]] = 1
        vm = np.broadcast_to(valid.astype(ml_dtypes.bfloat16),
                             (T, NTOK)).copy()
        padr = (~valid).astype(ml_dtypes.bfloat16)[None, :]

        # gold-path table part (trans/start/end/b_out counts) on host
        gh = 0.0
        for b in range(BC):
            L = int(len_c[b])
            tg = tags_c[b, :L]
            gh += float(trans[tg[:-1], tg[1:]].sum())
            gh += float(start_t[tg[0]] + end_t[tg[-1]])
            gh += float(bout[tg].sum())
        gold_host_total += gh

        h0t = np.zeros((128, 2, 2, BC), dtype=ml_dtypes.bfloat16)
        c0t = np.zeros((128, 2, 2, BC), dtype=np.float32)
        for d in range(2):
            for k in range(2):
                h0t[:, d, k, :] = h0[d][bs][:, k * 128:(k + 1) * 128].T
                c0t[:, d, k, :] = c0[d][bs][:, k * 128:(k + 1) * 128].T

        in_maps.append(dict(
            xt=xt, wih=wih_pack, whh=whh_pack, biast=bias_pack,
            wout=wout_pack, h0t=h0t, c0t=c0t,
            mp=mp.astype(ml_dtypes.bfloat16),
            mpt=mp.T.copy().astype(ml_dtypes.bfloat16),
            eend=eend, bvec=bvec, ohm=ohm, vmask=vm, padrow=padr,
        ))

    return in_maps, dict(kappa=kappa, k_len_total=k_len_total,
                         gold_host_total=gold_host_total)


def finalize(results, host):
    logz = sum(float(r["out"][0, 0]) for r in results)
    gold_em = sum(float(r["out"][0, 1]) for r in results)
    logz += host["kappa"] * host["k_len_total"]
    score = gold_em + host["gold_host_total"]
    return np.float32((logz - score) / B)


# ---------------------------------------------------------------- entry point
_COMPILED = {}


def kernel(**inputs):
    """Full-input BiLSTM-CRF loss on 8 NeuronCores (data parallel)."""
    from concourse.bass_utils import run_bass_kernel_spmd
    in_maps, host = prep_inputs(inputs)
    if "nc" not in _COMPILED:
        _COMPILED["nc"] = build_nc()
    nc = _COMPILED["nc"]
    res = run_bass_kernel_spmd(nc, in_maps, core_ids=list(range(N_CORES)))
    return np.asarray(finalize(res.results, host))


# revision 43
# speedup vs baseline: 1.0146x; 1.0013x over previous
"""BiLSTM-CRF loss kernel for Trainium2, 8-core data parallel.

Per-core (batch shard of 32), feature-major ("transposed") layout throughout:
gates/features live on partitions, batch on the free dim, so every elementwise
op runs at 128-partition occupancy with a small free size.

  - z_t for each direction accumulates in PSUM as [128 gates-in-chunk,
    8 chunks, batch]: per (chunk, dir) group = 1 bias matmul (K=1 ones rhs)
    + 4 x-projection matmuls (xg in [E, token] layout, consumed in-loop; no
    DRAM z roundtrip) + 2 recurrent matmuls off the transposed h buffer.
  - one sigmoid covers all 8 gate chunks; the g-gate rows of W/b are
    host-prescaled by 2 so tanh(g) = 2*sigmoid(z_g) - 1, done as a single
    DVE scalar_tensor_tensor. f*c runs on GpSimd (Pool) off the DVE path.
  - h = sigma_o * tanh(c) is written directly into the persistent transposed
    h buffer [128, k, dir, token] feeding both the next step's matmuls and
    the emission matmuls -- no PE transposes anywhere.
  - emissions (em = Wout.[hf;hb]) are computed per 512-token block as soon
    as both chains have covered it, with exp/vmask/gold-dot fused in.
  - CRF partition function in scaled linear space with an absorbing 77th
    tag: meet-in-the-middle (alpha forward 64 steps, beta/gamma backward 64
    steps, run concurrently), Z = alpha_63 . (M gamma_64).
Host combines the 8 per-core partial sums into the scalar loss.
"""

import numpy as np
import ml_dtypes

import concourse.bass as bass
import concourse.mybir as mybir
from concourse.tile import TileContext
from concourse.vector_clock import ScopedClock
from concourse.alu_op_type import AluOpType as ALU

N_CORES = 8
B, S, E, HD, T, V = 256, 128, 512, 256, 76, 30000
BC = B // N_CORES          # 32 batch per core
G4 = 4 * HD                # 1024 gates per direction
TA = T + 1                 # 77 tags with absorber
NTOK = S * BC              # 4096 tokens per direction per core

dt = mybir.dt
F32, BF16 = dt.float32, dt.bfloat16
AF = mybir.ActivationFunctionType

# ---------------------------------------------------------------- tile patch
# This walrus build rejects >1 sem wait on CTRL-class (Drain/NoOp)
# instructions; split the Tile tail-drain waits across preceding NOPs.
_MAX_WAITS = 1
_WAIT_LIMITS = {}


def _split_excess_waits(nc):
    """Non-DMA instructions accept only one sem wait on this walrus build;
    move excess waits onto NOPs spliced in front (same engine, same order)."""
    for f in nc.m.functions:
        stack = list(f.blocks)
        while stack:
            bb = stack.pop()
            for sub in getattr(bb, "blocks", []) or []:
                stack.append(sub)
            insts = getattr(bb, "instructions", None)
            if not insts:
                continue
            newlist = []
            changed = False
            for inst in insts:
                si = inst.sync_info
                lim = _WAIT_LIMITS.get(type(inst).__name__, 1)
                if si is not None and si.on_wait and len(si.on_wait) > lim:
                    waits = list(si.on_wait)
                    si.on_wait = waits[-lim:]
                    for w in waits[:-lim]:
                        nop = mybir.InstNoOp(
                            name=f"I-wsplit{nc.next_id()}", ins=[], outs=[],
                            engine=inst.engine,
                            sync_info=mybir.SyncInfo(on_wait=[w], on_update=[]),
                        )
                        newlist.append(nop)
                    changed = True
                newlist.append(inst)
            if changed:
                insts[:] = newlist


def _patched_drain_and_barrier(self, tick_clock, wait_clock):
    nc = self.nc
    _split_excess_waits(nc)
    nops = [nc.sync.nop(nofuse=True, hint=f"waitsplit{i}") for i in range(16)]
    drain_inst = nc.sync.drain()
    wait_clock.add_sem_waits(
        drain_inst.ins, ScopedClock({None: tick_clock.global_clock})
    )
    si = drain_inst.ins.sync_info
    if si is not None and si.on_wait and len(si.on_wait) > _MAX_WAITS:
        waits = list(si.on_wait)
        chunks = [waits[i:i + _MAX_WAITS] for i in range(0, len(waits), _MAX_WAITS)]
        si.on_wait = chunks[-1]
        assert len(chunks) - 1 <= len(nops), "too many wait chunks"
        for i, ch in enumerate(chunks[:-1]):
            ni = nops[i].ins
            if ni.sync_info is None:
                ni.sync_info = mybir.SyncInfo(on_wait=ch, on_update=[])
            else:
                ni.sync_info.on_wait = list(ni.sync_info.on_wait) + ch
    nc.all_engine_barrier()
    assert self.sems is not None
    popped = nc._tile_sem_poison_stack.pop()
    assert popped is self._sem_poison
    allsems = list(self.sems.allocated().values())
    for i in range(0, len(allsems), 8):
        nc.clear_and_free_semaphores(allsems[i:i + 8])
    nc.all_engine_barrier()


def apply_tile_patch():
    TileContext._drain_and_barrier = _patched_drain_and_barrier


# ---------------------------------------------------------------- builder
def build_nc():
    apply_tile_patch()
    nc = bass.Bass("TRN2", target_bir_lowering=False, debug=False,
                   num_devices=N_CORES)

    xt_d = nc.dram_tensor("xt", [2, 128, 4, NTOK], BF16, kind="ExternalInput")
    wih_d = nc.dram_tensor("wih", [128, 2, 4, G4], BF16, kind="ExternalInput")
    whh_d = nc.dram_tensor("whh", [128, 2, 2, G4], BF16, kind="ExternalInput")
    bias_d = nc.dram_tensor("biast", [1, 2, 8, 128], BF16, kind="ExternalInput")
    wout_d = nc.dram_tensor("wout", [128, 2, 2, T], BF16, kind="ExternalInput")
    h0t_d = nc.dram_tensor("h0t", [128, 2, 2, BC], BF16, kind="ExternalInput")
    c0t_d = nc.dram_tensor("c0t", [128, 2, 2, BC], F32, kind="ExternalInput")
    mp_d = nc.dram_tensor("mp", [TA, TA], BF16, kind="ExternalInput")
    mpt_d = nc.dram_tensor("mpt", [TA, TA], BF16, kind="ExternalInput")
    eend_d = nc.dram_tensor("eend", [TA, 1], F32, kind="ExternalInput")
    bvec_d = nc.dram_tensor("bvec", [T, 2], F32, kind="ExternalInput")
    ohm_d = nc.dram_tensor("ohm", [T, NTOK], BF16, kind="ExternalInput")
    vmask_d = nc.dram_tensor("vmask", [T, NTOK], BF16, kind="ExternalInput")
    padrow_d = nc.dram_tensor("padrow", [1, NTOK], BF16, kind="ExternalInput")
    out_d = nc.dram_tensor("out", [1, 2], F32, kind="ExternalOutput")

    NB = S // 16  # 8 emission blocks of 512 tokens
    # slot (0-based) after which an emission column range is available; the
    # backward chain is software-pipelined two slots behind the forward one.
    # Block 0 splits: cols 32:512 need b-step 126 (slot 128); cols 0:32 need
    # the very last b-step 127 (slot 129) and are kept tiny to start the CRF
    # sooner.
    em_ready = {}
    for b in range(NB):
        r = max(16 * b + 15, S - 1 - 16 * b + 2)
        if b == 0:
            em_ready.setdefault(r - 1, []).append((0, 32, 512))
            em_ready.setdefault(r, []).append((0, 0, 32))
        else:
            em_ready.setdefault(r, []).append((b, 0, 512))

    with TileContext(nc) as tc:
        with (
            tc.tile_pool(name="const", bufs=1) as cpool,
            tc.tile_pool(name="hbuf", bufs=1) as hpool,
            tc.tile_pool(name="gate", bufs=3) as gpool,
            tc.tile_pool(name="cell", bufs=3) as spool,
            tc.tile_pool(name="work", bufs=3) as wpool,
            tc.tile_pool(name="zps", bufs=2, space="PSUM") as zps_pool,
            tc.tile_pool(name="emps", bufs=1, space="PSUM") as emps_pool,
            tc.tile_pool(name="crfps", bufs=2, space="PSUM") as crfps_pool,
        ):
            # ---- constants / weights into SBUF.  DMA order is the startup
            # critical path: everything step 0 needs (bias, h0, c0, wih,
            # whh, first xg chunk) goes first; the rest streams in behind.
            bias_sb = cpool.tile([1, 2, 8, 128], BF16)
            nc.sync.dma_start(bias_sb[:], bias_d[:])
            h0t_sb = cpool.tile([128, 2, 2, BC], BF16)
            nc.sync.dma_start(h0t_sb[:], h0t_d[:])
            c0t_sb = cpool.tile([128, 2, 2, BC], F32)
            nc.sync.dma_start(c0t_sb[:], c0t_d[:])
            wih_sb = cpool.tile([128, 2, 4, G4], BF16)
            nc.sync.dma_start(wih_sb[:, 0], wih_d.ap()[:, 0])
            whh_sb = cpool.tile([128, 2, 2, G4], BF16)
            nc.sync.dma_start(whh_sb[:], whh_d[:])
            xg = {d: hpool.tile([128, 4, NTOK], BF16, name=f"xg{d}")
                  for d in range(2)}
            NCH = 4
            CW = NTOK // NCH
            nc.sync.dma_start(xg[0][:, :, 0:CW], xt_d.ap()[0, :, :, 0:CW])
            nc.sync.dma_start(wih_sb[:, 1], wih_d.ap()[:, 1])
            nc.sync.dma_start(xg[1][:, :, 0:CW], xt_d.ap()[1, :, :, 0:CW])

            wout_sb = cpool.tile([128, 2, 2, T], BF16)
            nc.sync.dma_start(wout_sb[:], wout_d[:])
            mp_sb = cpool.tile([TA, TA], BF16)
            nc.sync.dma_start(mp_sb[:], mp_d[:])
            mpt_sb = cpool.tile([TA, TA], BF16)
            nc.sync.dma_start(mpt_sb[:], mpt_d[:])
            eend_sb = cpool.tile([TA, 1], F32)
            nc.sync.dma_start(eend_sb[:], eend_d[:])
            bvec_sb = cpool.tile([T, 2], F32)
            nc.sync.dma_start(bvec_sb[:], bvec_d[:])

            ones1 = cpool.tile([1, BC], BF16)
            nc.vector.memset(ones1[:], 1.0)
            onesd = cpool.tile([128, 2, BC], BF16)
            nc.vector.memset(onesd[:], 1.0)
            ones77 = cpool.tile([TA, 1], F32)
            nc.vector.memset(ones77[:], 1.0)

            # remaining xg chunks stream in behind the first ones
            for c in range(1, NCH):
                for d in range(2):
                    nc.sync.dma_start(
                        xg[d][:, :, c * CW:(c + 1) * CW],
                        xt_d.ap()[d, :, :, c * CW:(c + 1) * CW])
            # transposed h, one tile per direction: [128, k-chunk, token]
            hts = {d: hpool.tile([128, 2, NTOK], BF16, name=f"hts{d}")
                   for d in range(2)}
            # emissions (scaled-exp'd), bf16, absorber row 76
            em_sb = hpool.tile([TA, NTOK], BF16, name="em")
            ohm_sb = hpool.tile([T, NTOK], BF16, name="ohm")
            nc.sync.dma_start(ohm_sb[:], ohm_d[:])
            vm_sb = hpool.tile([T, NTOK], BF16, name="vm")
            nc.sync.dma_start(vm_sb[:], vmask_d[:])
            nc.sync.dma_start(em_sb[T:TA, :], padrow_d[:])

            # ---- z PSUM tile helpers -----------------------------------
            def emit_bias_x(zt, d, t):
                """bias + x-projection matmuls of direction d for step t into
                PSUM tile zt [128, 8 gate-chunk, BC]."""
                tok = slice(t * BC, (t + 1) * BC)
                for gc in range(8):
                    nc.tensor.matmul(
                        zt[:, gc, :], bias_sb[:, d, gc, :],
                        ones1[:], start=True, stop=False)
                for ek in range(4):
                    for gc in range(8):
                        nc.tensor.matmul(
                            zt[:, gc, :],
                            wih_sb[:, d, ek, gc * 128:(gc + 1) * 128],
                            xg[d][:, ek, tok], start=False, stop=False)

            def emit_h(zt, d, t):
                """recurrent matmuls (Whh . h_{t-1}) closing step t's groups."""
                for k in range(2):
                    if t == 0:
                        hk = h0t_sb[:, d, k, :]
                    else:
                        col = (t - 1 if d == 0 else S - t) * BC
                        hk = hts[d][:, k, col:col + BC]
                    for gc in range(8):
                        nc.tensor.matmul(
                            zt[:, gc, :],
                            whh_sb[:, d, k, gc * 128:(gc + 1) * 128],
                            hk, start=False, stop=(k == 1))

            # ---- emission block -----------------------------------------
            em_accs = []

            def emit_emission(b, lo, hi):
                blk = slice(b * 512 + lo, b * 512 + hi)
                w = hi - lo
                psf = emps_pool.tile([T, 512], F32, tag="emps", name="emps")
                ps = psf[:, 0:w]
                i = 0
                for d in range(2):
                    for k in range(2):
                        nc.tensor.matmul(ps, wout_sb[:, k, d, :],
                                         hts[d][:, k, blk],
                                         start=(i == 0), stop=(i == 3))
                        i += 1
                # gold-path dot on raw em (b_out part handled on host)
                scr = wpool.tile([T, w], BF16, tag=f"scr{b}_{lo}", bufs=1,
                                 name="scr")
                nc.vector.tensor_mul(scr[:], ps, ohm_sb[:, blk])
                acc = wpool.tile([T, 1], F32, tag=f"emacc{b}_{lo}", bufs=1,
                                 name="emacc")
                em_accs.append(acc)
                nc.vector.tensor_reduce(acc[:], scr[:],
                                        axis=mybir.AxisListType.X,
                                        op=ALU.add)
                # scaled emissions: exp(em + b_out [+ start_trans at t=0])
                bias = bvec_sb[:, 1:2] if (b == 0 and lo == 0) \
                    else bvec_sb[:, 0:1]
                nc.scalar.activation(em_sb[0:T, blk], ps, AF.Exp,
                                     bias=bias)
                if not (b == 0 and lo == 0):  # t=0 is always valid
                    nc.vector.tensor_mul(em_sb[0:T, blk], em_sb[0:T, blk],
                                         vm_sb[:, blk])

            # ---- LSTM loop ----------------------------------------------
            # Forward chain runs in slot t = its step t; the backward chain
            # is software-pipelined one slot behind (step t in slot t+1) so
            # its Act/DVE ops always have ready inputs and can never stall
            # the forward chain through the in-order engine queues.
            c_st = {d: c0t_sb[:, d, :, :] for d in range(2)}

            def sig_phase(d, zt):
                g = gpool.tile([128, 8, BC], BF16, tag=f"g{d}", name=f"g{d}")
                nc.scalar.activation(g[:], zt[:], AF.Sigmoid)
                return g

            def dve_phase(d, g):
                fc = spool.tile([128, 2, BC], F32, tag=f"fc{d}",
                                name=f"fc{d}")
                nc.gpsimd.tensor_mul(fc[:], g[:, 2:4, :], c_st[d])
                tg = spool.tile([128, 2, BC], BF16, tag=f"tg{d}",
                                name=f"tg{d}")
                # tanh(g) = 2*sigmoid(2g) - 1 (g-rows prescaled by 2)
                nc.vector.scalar_tensor_tensor(
                    tg[:], g[:, 6:8, :], 2.0, onesd[:],
                    op0=ALU.mult, op1=ALU.subtract)
                ig = spool.tile([128, 2, BC], BF16, tag=f"ig{d}",
                                name=f"ig{d}")
                nc.vector.tensor_mul(ig[:], tg[:], g[:, 0:2, :])
                cn = spool.tile([128, 2, BC], F32, tag=f"c{d}", name=f"c{d}")
                nc.vector.tensor_add(cn[:], fc[:], ig[:])
                return cn

            def tanh_phase(d, cn):
                th = spool.tile([128, 2, BC], BF16, tag=f"th{d}",
                                name=f"th{d}")
                nc.scalar.activation(th[:], cn[:], AF.Tanh)
                return th

            def hm_phase(d, t, g, cn, th):
                col = (t if d == 0 else S - 1 - t) * BC
                nc.vector.tensor_mul(hts[d][:, :, col:col + BC],
                                     g[:, 4:6, :], th[:])
                c_st[d] = cn[:]

            def new_z(d):
                return zps_pool.tile([128, 8, BC], F32, tag=f"z{d}",
                                     name=f"z{d}")

            # backward chain runs D slots behind the forward chain so its
            # Act/DVE ops always have slot-old inputs and never stall the
            # forward chain through the in-order engine queues
            D = 2
            zcur = {0: new_z(0)}
            emit_bias_x(zcur[0], 0, 0)
            for slot in range(S + D):
                fon = slot < S
                bon = slot >= D
                tb_ = slot - D
                if bon:
                    emit_h(zcur[1], 1, tb_)
                if fon:
                    emit_h(zcur[0], 0, slot)
                gf = sig_phase(0, zcur[0]) if fon else None
                gb = sig_phase(1, zcur[1]) if bon else None
                cf = dve_phase(0, gf) if fon else None
                cb = dve_phase(1, gb) if bon else None
                tf = tanh_phase(0, cf) if fon else None
                tbh = tanh_phase(1, cb) if bon else None
                if fon:
                    hm_phase(0, slot, gf, cf, tf)
                if bon:
                    hm_phase(1, tb_, gb, cb, tbh)
                if slot < S - 1:
                    zf = new_z(0)
                    emit_bias_x(zf, 0, slot + 1)
                    zcur[0] = zf
                if 0 <= slot - D + 1 < S:
                    zb = new_z(1)
                    emit_bias_x(zb, 1, slot - D + 1)
                    zcur[1] = zb
                for b, lo, hi in em_ready.get(slot, []):
                    emit_emission(b, lo, hi)

            # ---- CRF: meet-in-the-middle forward/backward ---------------
            # emitted in bursts of 4 steps per chain to amortize the
            # cross-chain head-of-line coupling on the in-order engines
            half = S // 2  # alpha covers em 0..63, gamma covers 127..64
            a_prev = em_sb[:, 0:BC]
            gma = gpool.tile([TA, BC], BF16, tag="gma", name="gma")
            nc.vector.tensor_scalar_mul(
                gma[:], em_sb[:, (S - 1) * BC:S * BC], eend_sb[:])
            g_prev = gma[:]

            def alpha_step(i):
                nonlocal a_prev
                ta_ = i + 1
                aps = crfps_pool.tile([TA, BC], F32, tag="crf")
                nc.tensor.matmul(aps[:], mp_sb[:], a_prev,
                                 start=True, stop=True)
                a_new = gpool.tile([TA, BC], BF16, tag="a", name="a")
                nc.vector.tensor_mul(
                    a_new[:], aps[:], em_sb[:, ta_ * BC:(ta_ + 1) * BC])
                a_prev = a_new[:]

            def gamma_step(i):
                nonlocal g_prev
                tb_ = S - 2 - i
                gps = crfps_pool.tile([TA, BC], F32, tag="crf")
                nc.tensor.matmul(gps[:], mpt_sb[:], g_prev,
                                 start=True, stop=True)
                g_new = gpool.tile([TA, BC], BF16, tag="gma", name="gma")
                nc.vector.tensor_mul(
                    g_new[:], gps[:], em_sb[:, tb_ * BC:(tb_ + 1) * BC])
                g_prev = g_new[:]

            for i in range(half - 1):
                alpha_step(i)
                gamma_step(i)

            # Z = alpha_63 . (M gamma_64)
            wps = crfps_pool.tile([TA, BC], F32, tag="crf")
            nc.tensor.matmul(wps[:], mpt_sb[:], g_prev, start=True, stop=True)
            u = wpool.tile([TA, BC], F32, tag="u")
            nc.vector.tensor_mul(u[:], wps[:], a_prev)
            zsc = crfps_pool.tile([1, BC + 8], F32, tag="zsc", bufs=1)
            nc.tensor.matmul(zsc[:, 0:BC], ones77[:], u[:],
                             start=True, stop=True)
            logs = wpool.tile([1, BC], F32, tag="logs")
            nc.scalar.activation(logs[:], zsc[:, 0:BC], AF.Ln)
            logsum = wpool.tile([1, 1], F32, tag="logsum")
            nc.vector.tensor_reduce(logsum[:], logs[:],
                                    axis=mybir.AxisListType.X, op=ALU.add)

            # ---- gold emission score sum --------------------------------
            tot = wpool.tile([T, 1], F32, tag="tot")
            nc.vector.tensor_add(tot[:], em_accs[0][:], em_accs[1][:])
            for acc in em_accs[2:]:
                nc.vector.tensor_add(tot[:], tot[:], acc[:])
            nc.tensor.matmul(zsc[:, BC:BC + 1], ones77[0:T, :], tot[:],
                             start=True, stop=True)

            res = wpool.tile([1, 2], F32, tag="res")
            nc.vector.tensor_copy(res[:, 0:1], logsum[:])
            nc.vector.tensor_copy(res[:, 1:2], zsc[:, BC:BC + 1])
            nc.sync.dma_start(out_d[:], res[:])

    return nc


# ---------------------------------------------------------------- host side
def _gate_perm():
    """PyTorch gate order i,f,g,o -> reordered i,f,o,g (rows of W/b)."""
    return np.concatenate([
        np.arange(0, HD),            # i
        np.arange(HD, 2 * HD),       # f
        np.arange(3 * HD, 4 * HD),   # o
        np.arange(2 * HD, 3 * HD),   # g
    ])


def _pack_w_t(w, perm, nchunks, gscale):
    """w: [G4, kdim] -> [128, nchunks, G4] bf16 with
    out[p, c, g] = w[perm[g], c*128+p] * gscale[g]."""
    wp = np.asarray(w, dtype=np.float32)[perm, :] * gscale[:, None]
    out = np.empty((128, nchunks, G4), dtype=ml_dtypes.bfloat16)
    for c in range(nchunks):
        out[:, c, :] = wp[:, c * 128:(c + 1) * 128].T.astype(ml_dtypes.bfloat16)
    return out


def prep_inputs(inputs):
    """Build per-core input maps + host constants."""
    ids = np.asarray(inputs["input_ids"])
    tags = np.asarray(inputs["tag_ids"])
    lengths = np.asarray(inputs["lengths"])
    perm = _gate_perm()
    # gate g (index 768:1024 after perm) prescaled by 2 for the
    # tanh(x) = 2*sigmoid(2x)-1 identity
    gscale = np.ones(G4, dtype=np.float32)
    gscale[3 * HD:] = 2.0

    embed_bf = np.asarray(inputs["embed_table"]).astype(ml_dtypes.bfloat16)

    def gather_xt(flat_ids):
        g = embed_bf[flat_ids]                       # [NTOK, E] bf16
        return np.ascontiguousarray(
            g.reshape(NTOK, 4, 128).transpose(2, 1, 0))

    wih_pack = np.stack([_pack_w_t(inputs["W_ih_f"], perm, 4, gscale),
                         _pack_w_t(inputs["W_ih_b"], perm, 4, gscale)],
                        axis=1)                      # [128, 2, 4, G4]
    whh_pack = np.stack([_pack_w_t(inputs["W_hh_f"], perm, 2, gscale),
                         _pack_w_t(inputs["W_hh_b"], perm, 2, gscale)],
                        axis=1)                      # [128, 2, 2, G4]
    bias_f = ((np.asarray(inputs["b_ih_f"]) + np.asarray(inputs["b_hh_f"]))
              [perm] * gscale)
    bias_b = ((np.asarray(inputs["b_ih_b"]) + np.asarray(inputs["b_hh_b"]))
              [perm] * gscale)
    bias_pack = np.stack([bias_f.reshape(8, 128), bias_b.reshape(8, 128)]
                         )[None].astype(ml_dtypes.bfloat16)  # [1, 2, 8, 128]

    wo = np.asarray(inputs["W_out"])                 # [T, H]
    wout_pack = np.empty((128, 2, 2, T), dtype=ml_dtypes.bfloat16)
    for d in range(2):
        for k in range(2):
            sl = slice(d * 256 + k * 128, d * 256 + (k + 1) * 128)
            wout_pack[:, k, d, :] = wo[:, sl].T.astype(ml_dtypes.bfloat16)

    trans = np.asarray(inputs["trans"]).astype(np.float64)
    start_t = np.asarray(inputs["start_trans"]).astype(np.float64)
    end_t = np.asarray(inputs["end_trans"]).astype(np.float64)
    bout = np.asarray(inputs["b_out"]).astype(np.float64)
    kappa = float(np.log(np.exp(trans).sum(axis=0).mean()))

    mp = np.zeros((TA, TA), dtype=np.float64)
    mp[0:T, 0:T] = np.exp(trans - kappa)
    mp[0:T, T] = np.exp(end_t - kappa)
    mp[T, T] = 1.0
    eend = np.zeros((TA, 1), dtype=np.float32)
    eend[0:T, 0] = np.exp(end_t)
    eend[T, 0] = 1.0
    bvec = np.zeros((T, 2), dtype=np.float32)
    bvec[:, 0] = bout
    bvec[:, 1] = bout + start_t

    h0 = np.asarray(inputs["h0"])                    # [2, B, HD]
    c0 = np.asarray(inputs["c0"])

    in_maps = []
    k_len_total = 0
    gold_host_total = 0.0
    for c in range(N_CORES):
        bs = slice(c * BC, (c + 1) * BC)
        ids_c = ids[bs]
        tags_c = tags[bs]
        len_c = lengths[bs].astype(np.int64)
        k_len_total += int(np.minimum(len_c, S - 1).sum())

        idx_f = ids_c.T.reshape(-1)                    # token (s, b) order
        idx_b = ids_c[:, ::-1].T.reshape(-1)
        xt = np.stack([gather_xt(idx_f), gather_xt(idx_b)])

        svec = np.arange(S)[None, :]
        valid = (svec < len_c[:, None]).T.reshape(-1)  # [(s, b)]
        ohm = np.zeros((T, NTOK), dtype=ml_dtypes.bfloat16)
        tt = tags_c.T.reshape(-1)
        pos = np.arange(NTOK)
        ohm[tt[valid], pos[valid]] = 1
        vm = np.broadcast_to(valid.astype(ml_dtypes.bfloat16),
                             (T, NTOK)).copy()
        padr = (~valid).astype(ml_dtypes.bfloat16)[None, :]

        # gold-path table part (trans/start/end/b_out counts) on host
        gh = 0.0
        for b in range(BC):
            L = int(len_c[b])
            tg = tags_c[b, :L]
            gh += float(trans[tg[:-1], tg[1:]].sum())
            gh += float(start_t[tg[0]] + end_t[tg[-1]])
            gh += float(bout[tg].sum())
        gold_host_total += gh

        h0t = np.zeros((128, 2, 2, BC), dtype=ml_dtypes.bfloat16)
        c0t = np.zeros((128, 2, 2, BC), dtype=np.float32)
        for d in range(2):
            for k in range(2):
                h0t[:, d, k, :] = h0[d][bs][:, k * 128:(k + 1) * 128].T
                c0t[:, d, k, :] = c0[d][bs][:, k * 128:(k + 1) * 128].T

        in_maps.append(dict(
            xt=xt, wih=wih_pack, whh=whh_pack, biast=bias_pack,
            wout=wout_pack, h0t=h0t, c0t=c0t,
            mp=mp.astype(ml_dtypes.bfloat16),
            mpt=mp.T.copy().astype(ml_dtypes.bfloat16),
            eend=eend, bvec=bvec, ohm=ohm, vmask=vm, padrow=padr,
        ))

    return in_maps, dict(kappa=kappa, k_len_total=k_len_total,
                         gold_host_total=gold_host_total)


def finalize(results, host):
    logz = sum(float(r["out"][0, 0]) for r in results)
    gold_em = sum(float(r["out"][0, 1]) for r in results)
    logz += host["kappa"] * host["k_len_total"]
    score = gold_em + host["gold_host_total"]
    return np.float32((logz - score) / B)


# ---------------------------------------------------------------- entry point
_COMPILED = {}


def kernel(**inputs):
    """Full-input BiLSTM-CRF loss on 8 NeuronCores (data parallel)."""
    from concourse.bass_utils import run_bass_kernel_spmd
    in_maps, host = prep_inputs(inputs)
    if "nc" not in _COMPILED:
        _COMPILED["nc"] = build_nc()
    nc = _COMPILED["nc"]
    res = run_bass_kernel_spmd(nc, in_maps, core_ids=list(range(N_CORES)))
    return np.asarray(finalize(res.results, host))
